# revision 1
# baseline (speedup 1.0000x reference)
"""MoE LoRA linear kernel for Trainium2, 8 NeuronCores, data-parallel over tokens.

Reference computation (per token x, D=4096, E=28 experts, rank 8, top-2):
  base   = x @ W^T
  logits = x @ gate_W^T ; top-2 softmax -> per-expert gates g (0 elsewhere)
  h_e    = x @ A_e^T                     (all experts, rank 8)
  out    = base + sum_e g_e*2 * h_e @ B_e^T

Sharding: tokens split 8 ways (1024 tokens/core); weights replicated.

Numerics: big GEMMs in fp32r (fp32 with 12-bit mantissa, full PE rate at
N>=256); weights pre-rounded on host, x rounded on device by the DVE copy
into the fp32r-typed resident tile.  Gate logits run in full fp32 so top-2
selection matches the fp32 reference.

Structure (per core):
  prologue: stream x (fp32) for token tiles 0-3; each streamed tile feeds
            (a) exact-fp32 gate matmuls and (b) a DVE cast into the resident
            fp32r x; batched top-2 softmax for tiles 0-3; lora-h + transpose
            for tiles 0-3 -> h'T[:, 0:512].
  main:     11 output-column groups (3x128 rows, last 2x128) x 32 k-tiles of
            the base GEMM, streaming W once; token tiles 4-7 of the gating/
            lora-h pipeline are emitted inside group 0's k-loop so their DMA
            hides under matmuls; each group finishes with the 4 lora rank-
            combine matmuls accumulated into the same PSUM, copy-out, store.
"""
import sys

for _p in ("/opt/trn_rl_repo", "/root/.axon_site/_ro/trn_rl_repo"):
    if _p not in sys.path:
        sys.path.insert(0, _p)

import numpy as np

import concourse.bass as bass
import concourse.mybir as mybir
import concourse.tile as tile
from concourse import bacc, bass_utils
from concourse.masks import make_identity

F32 = mybir.dt.float32
F32R = mybir.dt.float32r

N_CORES = 8
B, S, D_IN, D_OUT = 4, 2048, 4096, 4096
N_EXPERTS, RANK, SCALING = 28, 8, 2.0
ER = N_EXPERTS * RANK          # 224
T = B * S // N_CORES           # 1024 tokens per core
P = 128
KT = D_IN // P                 # 32 k-tiles
JT = D_OUT // P                # 32 output row-tiles
TT = T // P                    # 8 token tiles
NCH = 512                      # moving free dim chunk
TCH = T // NCH                 # 2 token chunks of 512
JG = 3                         # j-tiles per psum group (3x2 chunks = 6 banks)
AT_COLS = 256                  # 224 used + 32 zero pad (N>=256 keeps fp32r fast)


def round_fp32r(x: np.ndarray) -> np.ndarray:
    """Round fp32 to the fp32r (12-bit mantissa) grid, matching PE behavior."""
    u = np.ascontiguousarray(x).view(np.uint32).astype(np.uint64)
    r = ((u + 0x800) & ~np.uint64(0xFFF)).astype(np.uint32)
    return r.view(np.float32).reshape(x.shape)


def build_nc():
    nc = bacc.Bacc("TRN2", target_bir_lowering=False, debug=False)
    xg_d = nc.dram_tensor("xg", [TT, KT // 4, P, 4 * P], F32,
                          kind="ExternalInput").ap()
    wt_d = nc.dram_tensor("wt", [D_IN, D_OUT], F32R, kind="ExternalInput").ap()
    gw_d = nc.dram_tensor("gw", [P, KT * N_EXPERTS], F32,
                          kind="ExternalInput").ap()
    at_d = nc.dram_tensor("at", [P, KT * AT_COLS], F32R,
                          kind="ExternalInput").ap()
    ba_d = nc.dram_tensor("ba", [P, D_OUT], F32R, kind="ExternalInput").ap()
    bb_d = nc.dram_tensor("bb", [P, D_OUT], F32R, kind="ExternalInput").ap()
    out_d = nc.dram_tensor("out", [D_OUT, T], F32, kind="ExternalOutput").ap()

    gw_re = gw_d.rearrange("p (kt e) -> p kt e", kt=KT)
    at_re = at_d.rearrange("p (kt c) -> p kt c", kt=KT)

    # j-groups: group 0 runs token-chunk 0 only (its chunk-1 inputs are
    # produced by the phase-1 work interleaved into its k-loop); the deferred
    # chunk-1 half of group 0's rows runs as a final extra group.
    groups = [(0, JG, (0,))]
    groups += [(g * JG, JG, (0, 1)) for g in range(1, JT // JG)]
    if JT % JG:
        groups.append((JT - JT % JG, JT % JG, (0, 1)))
    groups.append((0, JG, (1,)))

    with tile.TileContext(nc) as tc:
        with (
            tc.tile_pool(name="resident", bufs=1) as rp,
            tc.tile_pool(name="wstream", bufs=6) as wp,
            tc.tile_pool(name="bstream", bufs=2) as bp,
            tc.tile_pool(name="xgstream", bufs=4) as xgp,
            tc.tile_pool(name="outstage", bufs=3) as op_,
            tc.tile_pool(name="smalls", bufs=2) as sp,
            tc.tile_pool(name="gating", bufs=1) as gp,
            tc.tile_pool(name="ph1ps", bufs=2, space="PSUM") as ph1,
            tc.tile_pool(name="psmm", bufs=6, space="PSUM") as psm,
        ):
            gw_sb = rp.tile([P, KT, N_EXPERTS], F32)
            nc.sync.dma_start(gw_sb[:], gw_re[:])
            ident = rp.tile([P, P], F32)
            make_identity(nc, ident[:])
            at_sb = rp.tile([P, KT, AT_COLS], F32R)
            nc.sync.dma_start(at_sb[:], at_re[:])
            xt_sb = rp.tile([P, KT, T], F32R)
            hta_sb = rp.tile([P, T], F32R)
            htb_sb = rp.tile([P, T], F32R)
            logits_all = rp.tile([P, TT, N_EXPERTS], F32)
            gsc_all = rp.tile([P, TT, AT_COLS // RANK], F32)

            def gate_tile(t):
                """Stream x tile t (fp32): gate matmuls + fp32r cast into
                resident x."""
                ts_ = slice(t * P, (t + 1) * P)
                pg = ph1.tile([P, N_EXPERTS], F32, name="pg", tag="ph1")
                for kq in range(KT // 4):
                    xg_t = xgp.tile([P, 4 * P], F32, name="xg_t")
                    nc.sync.dma_start(xg_t[:], xg_d[t, kq])
                    nc.vector.tensor_copy(
                        xt_sb[:, kq * 4:(kq + 1) * 4, ts_],
                        xg_t[:].rearrange("p (f t) -> p f t", f=4))
                    for k4 in range(4):
                        kt = kq * 4 + k4
                        nc.tensor.matmul(
                            pg[:], xg_t[:, k4 * P:(k4 + 1) * P], gw_sb[:, kt],
                            start=(kt == 0), stop=(kt == KT - 1))
                nc.vector.tensor_copy(logits_all[:, t], pg[:])

            def gate_chain(lo, hi):
                """Batched top-2 softmax for token tiles [lo, hi)."""
                n = hi - lo
                sl = slice(lo, hi)
                EB = (P, n, N_EXPERTS)
                m1 = gp.tile([P, n], F32, name=f"m1_{lo}", tag="m1")
                nc.vector.reduce_max(m1[:], logits_all[:, sl],
                                     axis=mybir.AxisListType.X)
                m1b = m1[:, :, None].to_broadcast(EB)
                eq = gp.tile([P, n, N_EXPERTS], F32, name=f"eq_{lo}", tag="eq")
                nc.vector.tensor_tensor(eq[:], logits_all[:, sl], m1b,
                                        mybir.AluOpType.is_equal)
                nc.vector.scalar_tensor_tensor(
                    eq[:], eq[:], -1e30, logits_all[:, sl],
                    mybir.AluOpType.mult, mybir.AluOpType.add)
                m2 = gp.tile([P, n], F32, name=f"m2_{lo}", tag="m2")
                nc.vector.reduce_max(m2[:], eq[:], axis=mybir.AxisListType.X)
                mask2 = gp.tile([P, n, N_EXPERTS], F32, name=f"mask2_{lo}",
                                tag="mask2")
                nc.vector.tensor_tensor(mask2[:], logits_all[:, sl],
                                        m2[:, :, None].to_broadcast(EB),
                                        mybir.AluOpType.is_ge)
                d1 = gp.tile([P, n, N_EXPERTS], F32, name=f"d1_{lo}", tag="d1")
                nc.vector.tensor_tensor(d1[:], logits_all[:, sl], m1b,
                                        mybir.AluOpType.subtract)
                nc.scalar.activation(d1[:], d1[:],
                                     mybir.ActivationFunctionType.Exp)
                d2 = gp.tile([P, n], F32, name=f"d2_{lo}", tag="d2")
                nc.vector.tensor_tensor(d2[:], m2[:], m1[:],
                                        mybir.AluOpType.subtract)
                nc.scalar.activation(d2[:], d2[:],
                                     mybir.ActivationFunctionType.Exp)
                nc.vector.tensor_scalar_add(d2[:], d2[:], 1.0)
                nc.vector.reciprocal(d2[:], d2[:])
                nc.vector.tensor_scalar_mul(d2[:], d2[:], SCALING)
                nc.vector.memset(gsc_all[:, sl, N_EXPERTS:], 0.0)
                nc.vector.tensor_tensor(d1[:], d1[:], mask2[:],
                                        mybir.AluOpType.mult)
                nc.vector.tensor_tensor(gsc_all[:, sl, :N_EXPERTS], d1[:],
                                        d2[:, :, None].to_broadcast(EB),
                                        mybir.AluOpType.mult)

            def h_tile(t):
                """lora h matmuls + gate multiply + transpose for tile t."""
                ts_ = slice(t * P, (t + 1) * P)
                ph_ = ph1.tile([P, AT_COLS], F32, name="ph", tag="ph1")
                for kt in range(KT):
                    nc.tensor.matmul(ph_[:], xt_sb[:, kt, ts_], at_sb[:, kt],
                                     start=(kt == 0), stop=(kt == KT - 1))
                hp = sp.tile([P, AT_COLS], F32, name="hp")
                nc.vector.tensor_tensor(
                    hp[:].rearrange("p (e r) -> p e r", r=RANK),
                    ph_[:].rearrange("p (e r) -> p e r", r=RANK),
                    gsc_all[:, t, :, None].to_broadcast(
                        (P, AT_COLS // RANK, RANK)),
                    mybir.AluOpType.mult)
                for half, dst in ((0, hta_sb), (1, htb_sb)):
                    pt = ph1.tile([P, P], F32, name="pt", tag="ph1")
                    nc.tensor.transpose(
                        pt[:], hp[:, half * P:(half + 1) * P], ident[:])
                    nc.vector.tensor_copy(dst[:, ts_], pt[:])

            # ---- prologue: token tiles 0-3 ----
            for t in range(TT // 2):
                gate_tile(t)
            gate_chain(0, TT // 2)
            for t in range(TT // 2):
                h_tile(t)

            # ---- main: base GEMM groups, tiles 4-7 interleaved in group 0 --
            hooks = {
                1: lambda: gate_tile(4),
                5: lambda: gate_tile(5),
                9: lambda: gate_tile(6),
                13: lambda: gate_tile(7),
                15: lambda: gate_chain(TT // 2, TT),
                17: lambda: h_tile(4),
                21: lambda: h_tile(5),
                25: lambda: h_tile(6),
                29: lambda: h_tile(7),
            }

            b_tiles = {}

            def load_b(gi):
                j0, nj, _ = groups[gi]
                js = slice(j0 * P, (j0 + nj) * P)
                ba_t = bp.tile([P, JG * P], F32R, name="ba_t")
                nc.sync.dma_start(ba_t[:, :nj * P], ba_d[:, js])
                bb_t = bp.tile([P, JG * P], F32R, name="bb_t")
                nc.sync.dma_start(bb_t[:, :nj * P], bb_d[:, js])
                b_tiles[gi] = (ba_t, bb_t)

            load_b(0)
            for gi, (j0, nj, chunks) in enumerate(groups):
                js = slice(j0 * P, (j0 + nj) * P)
                psums = {
                    (j, c): psm.tile([P, NCH], F32, name=f"pm_{j}_{c}", tag="pm")
                    for j in range(nj) for c in chunks
                }
                for kt in range(KT):
                    w_t = wp.tile([P, JG * P], F32R, name="w_t")
                    nc.sync.dma_start(
                        w_t[:, :nj * P], wt_d[kt * P:(kt + 1) * P, js])
                    if kt == 2 and gi + 1 < len(groups):
                        load_b(gi + 1)
                    for j in range(nj):
                        lhs = w_t[:, j * P:(j + 1) * P]
                        for c in chunks:
                            nc.tensor.matmul(
                                psums[j, c], lhs,
                                xt_sb[:, kt, c * NCH:(c + 1) * NCH],
                                start=(kt == 0), stop=False)
                    if gi == 0 and kt in hooks:
                        hooks[kt]()
                ba_t, bb_t = b_tiles.pop(gi)
                for j in range(nj):
                    for c in chunks:
                        cs = slice(c * NCH, (c + 1) * NCH)
                        nc.tensor.matmul(
                            psums[j, c], ba_t[:, j * P:(j + 1) * P],
                            hta_sb[:, cs], start=False, stop=False)
                        nc.tensor.matmul(
                            psums[j, c], bb_t[:, j * P:(j + 1) * P],
                            htb_sb[:, cs], start=False, stop=True)
                        ot = op_.tile([P, NCH], F32, name="ot")
                        nc.vector.tensor_copy(ot[:], psums[j, c])
                        nc.sync.dma_start(
                            out_d[(j0 + j) * P:(j0 + j + 1) * P, cs], ot[:])
    nc.compile()
    return nc


_NC_CACHE = None
_LAST_IN_MAPS = None


def _get_nc():
    global _NC_CACHE
    if _NC_CACHE is None:
        _NC_CACHE = build_nc()
    return _NC_CACHE


def kernel(x, base_W, gate_W, lora_A, lora_B):
    x = np.asarray(x, dtype=np.float32)
    base_W = np.asarray(base_W, dtype=np.float32)
    gate_W = np.asarray(gate_W, dtype=np.float32)
    lora_A = np.asarray(lora_A, dtype=np.float32)
    lora_B = np.asarray(lora_B, dtype=np.float32)

    xf = x.reshape(B * S, D_IN)
    wt_np = round_fp32r(np.ascontiguousarray(base_W.T))          # [D_in, D_out]
    # gw packed [P, KT*E]: gw[p, kt*E + e] = gate_W[e, kt*P + p]
    gw_np = np.ascontiguousarray(
        gate_W.T.reshape(KT, P, N_EXPERTS).transpose(1, 0, 2).reshape(
            P, KT * N_EXPERTS))
    # lora_A [E, R, D_in] -> at [(D_in), (e r)] padded, packed [P, KT*C]
    a_flat = lora_A.reshape(ER, D_IN)
    at_np = np.zeros((D_IN, AT_COLS), dtype=np.float32)
    at_np[:, :ER] = a_flat.T
    at_np = round_fp32r(np.ascontiguousarray(
        at_np.reshape(KT, P, AT_COLS).transpose(1, 0, 2).reshape(
            P, KT * AT_COLS)))
    # lora_B [E, D_out, R] -> b_flat [(e r), D_out] -> halves split at er=128
    b_flat = np.ascontiguousarray(
        lora_B.transpose(0, 2, 1).reshape(ER, D_OUT))
    ba_np = np.zeros((P, D_OUT), dtype=np.float32)
    bb_np = np.zeros((P, D_OUT), dtype=np.float32)
    ba_np[:] = b_flat[:P]
    bb_np[:ER - P] = b_flat[P:]
    ba_np = round_fp32r(ba_np)
    bb_np = round_fp32r(bb_np)

    in_maps = []
    for c in range(N_CORES):
        xc = np.ascontiguousarray(xf[c * T:(c + 1) * T].T)       # [D_in, T]
        # xg packed [TT, KT//4, P, 4P]: [t,kq,p,k4*P+tok] = xc[(kq*4+k4)*P+p, t*P+tok]
        xg_np = np.ascontiguousarray(
            xc.reshape(KT // 4, 4, P, TT, P).transpose(3, 0, 2, 1, 4).reshape(
                TT, KT // 4, P, 4 * P))
        in_maps.append({
            "xg": xg_np,
            "wt": wt_np,
            "gw": gw_np,
            "at": at_np,
            "ba": ba_np,
            "bb": bb_np,
        })

    global _LAST_IN_MAPS
    _LAST_IN_MAPS = in_maps
    nc = _get_nc()
    res = bass_utils.run_bass_kernel_spmd(nc, in_maps,
                                          core_ids=list(range(N_CORES)))
    out = np.empty((B * S, D_OUT), dtype=np.float32)
    for c in range(N_CORES):
        out[c * T:(c + 1) * T] = res.results[c]["out"].T
    return out.reshape(B, S, D_OUT)

